# revision 1
# baseline (speedup 1.0000x reference)
"""Causal self-attention (B=4, T=2048, D=1024, H=16) on 8 TRN2 NeuronCores.

Sharding: core c handles batch b=c//2 and head-group g=c%2 (8 heads).
Each core computes its heads' attention + a partial output projection
(contraction over its 512 attn channels); the host sums the two partials
per batch and adds b_out.

Per-core device kernel (all matmuls fp32r, transposed "channels on
partitions" layout):
  qk-proj   qkT[ch,T] = wqk.T @ xT          (ch-major, per head-pair m-chunks)
  v-proj    V[t,ch']  = xT.T @ wv_aug        (t-major, 65-wide per head: 64 v
                                              cols + a ones col for the softmax
                                              normalizer; bias via ones-row mm)
  rope      q',k' via DVE/gpsimd elementwise with host-built cos/sin tables
  S^T       [k,q] = k'^T q' per head, 2 heads packed in the PE array via
            tile_position row tiling (K=64 each)
  softmax   no-max-subtraction exp (score range validated ~|8|), causal mask
            added in PSUM on diagonal tiles, normalizer from the V ones col
  PV        attn_aug^T[65,q] = V_aug^T @ E^T accumulated over k blocks
  norm      attnT = attn_aug[0:64] * bcast(1/Z)
  out-proj  out[q,o] = attnT.T @ wo  (partial; host adds pair partials)
"""
import sys
import numpy as np

for _p in ("/opt/trn_rl_repo", "/root/.axon_site/_ro/trn_rl_repo"):
    if _p not in sys.path:
        sys.path.append(_p)

import concourse.bass as bass
import concourse.bacc as bacc
import concourse.tile as tile
import concourse.mybir as mybir
from concourse import bass_utils

F32 = mybir.dt.float32
F32R = mybir.dt.float32r
AF = mybir.ActivationFunctionType
ALU = mybir.AluOpType

B, T, D, H, DK = 4, 2048, 1024, 16, 64
NC_ = 8          # cores
HPG = 8          # heads per group
NPAIR = 4        # head pairs per core
KT = 8           # 128-row k-tiles over D
XC = 512         # x/qkv t-chunk width
NXC = T // XC    # 8
QC = 512         # attention q-chunk width
NQC = T // QC    # 4
NKB = T // 128   # 16 key blocks
MASK_VAL = -30000.0

_cache = {}


def _build_nc(trace_scopes=False):
    nc = bacc.Bacc("TRN2", target_bir_lowering=False, debug=False)

    xT_d = nc.dram_tensor("xT", [D, T], F32R, kind="ExternalInput").ap()
    wqk_d = nc.dram_tensor("wqk", [D, 1024], F32R, kind="ExternalInput").ap()
    wva_d = nc.dram_tensor("wva", [D, 520], F32R, kind="ExternalInput").ap()
    bva_d = nc.dram_tensor("bva", [1, 520], F32R, kind="ExternalInput").ap()
    ones_d = nc.dram_tensor("ones1", [1, 128], F32R, kind="ExternalInput").ap()
    wo_d = nc.dram_tensor("wo", [512, 1024], F32R, kind="ExternalInput").ap()
    bqk_d = nc.dram_tensor("bqk", [128, 8], F32, kind="ExternalInput").ap()
    cos_d = nc.dram_tensor("cos4", [128, T], F32, kind="ExternalInput").ap()
    sin_d = nc.dram_tensor("sin4", [128, T], F32, kind="ExternalInput").ap()
    out_d = nc.dram_tensor("out", [T, 1024], F32, kind="ExternalOutput").ap()

    with tile.TileContext(nc, pool_alloc_mode="queue") as tc:
        _emit(tc, nc, xT_d, wqk_d, wva_d, bva_d, ones_d, wo_d, bqk_d,
              cos_d, sin_d, out_d)
    nc.compile()
    return nc


def _emit(tc, nc, xT_d, wqk_d, wva_d, bva_d, ones_d, wo_d, bqk_d,
          cos_d, sin_d, out_d):
    from contextlib import ExitStack
    ctx = ExitStack()
    with ctx:
        consts = ctx.enter_context(tc.tile_pool(name="consts", bufs=1))
        vpool = ctx.enter_context(tc.tile_pool(name="vpool", bufs=1))
        qkp = ctx.enter_context(tc.tile_pool(name="qkp", bufs=8))
        ep = ctx.enter_context(tc.tile_pool(name="ep", bufs=5))
        zbp = ctx.enter_context(tc.tile_pool(name="zbp", bufs=2))
        atp = ctx.enter_context(tc.tile_pool(name="atp", bufs=16))
        ps_mm = ctx.enter_context(tc.tile_pool(name="ps_mm", bufs=2, space="PSUM"))
        ps_s = ctx.enter_context(tc.tile_pool(name="ps_s", bufs=2, space="PSUM"))
        ps_pv = ctx.enter_context(tc.tile_pool(name="ps_pv", bufs=2, space="PSUM"))

        # ---------------- constants ----------------
        cos_t = consts.tile([128, T], F32, tag="cos")
        nc.sync.dma_start(out=cos_t[:], in_=cos_d)
        sin_t = consts.tile([128, T], F32, tag="sin")
        nc.sync.dma_start(out=sin_t[:], in_=sin_d)
        bqk_t = consts.tile([128, 8], F32, tag="bqk")
        nc.sync.dma_start(out=bqk_t[:], in_=bqk_d)
        bva_t = consts.tile([1, 520], F32R, tag="bva")
        nc.sync.dma_start(out=bva_t[:], in_=bva_d)
        ones_t = consts.tile([1, 128], F32R, tag="ones")
        nc.sync.dma_start(out=ones_t[:], in_=ones_d)
        wva_t = consts.tile([128, KT, 520], F32R, tag="wva")
        nc.sync.dma_start(out=wva_t[:], in_=wva_d.rearrange("(k p) m -> p k m", p=128))
        # additive causal masks: tri block [128,128] (valid iff c-r>=0) and
        # the d=3 variant [128,256] = [all-masked | tri]
        mask_t = consts.tile([128, 128], F32, tag="mask")
        nc.gpsimd.memset(mask_t[:], 0.0)
        nc.gpsimd.affine_select(
            out=mask_t[:], in_=mask_t[:], compare_op=ALU.is_ge, fill=MASK_VAL,
            base=0, pattern=[[1, 128]], channel_multiplier=-1)
        mask3_t = consts.tile([128, 256], F32, tag="mask3")
        nc.gpsimd.memset(mask3_t[:, 0:128], MASK_VAL)
        nc.gpsimd.memset(mask3_t[:, 128:256], 0.0)
        nc.gpsimd.affine_select(
            out=mask3_t[:, 128:256], in_=mask3_t[:, 128:256], compare_op=ALU.is_ge,
            fill=MASK_VAL, base=0, pattern=[[1, 128]], channel_multiplier=-1)

        # V_aug for all 16 t-blocks: [128 tok, 16 * (8 heads * 65)]
        V_t = vpool.tile([128, NKB, 520], F32R, tag="V")

        xT_r = xT_d.rearrange("(k p) t -> p k t", p=128)
        wqk_r = wqk_d.rearrange("(k p) m -> p k m", p=128)

        at_tiles = []
        qkv_ctx = ExitStack()
        wqkp = qkv_ctx.enter_context(tc.tile_pool(name="wqkp", bufs=2))
        xp = qkv_ctx.enter_context(tc.tile_pool(name="xp", bufs=2))
        t1p = qkv_ctx.enter_context(tc.tile_pool(name="t1p", bufs=2))
        for p in range(NPAIR):
            # -------- load this pair's qk weight slice --------
            wqk_pair = wqkp.tile([128, KT, 256], F32R, tag="wqk")
            nc.sync.dma_start(out=wqk_pair[:], in_=wqk_r[:, :, 256 * p:256 * (p + 1)])

            qp_ts = [qkp.tile([128, QC], F32R, tag="qp", name=f"qp{p}_{i}") for i in range(NQC)]
            kp_ts = [qkp.tile([128, QC], F32R, tag="kp", name=f"kp{p}_{i}") for i in range(NQC)]

            for tq in range(NXC):
                c0 = tq * XC
                xc = xp.tile([128, KT, XC], F32R, tag="xc")
                nc.sync.dma_start(out=xc[:], in_=xT_r[:, :, c0:c0 + XC])

                if p == 0:
                    # ---- v-proj for the 2 t-blocks in this chunk ----
                    for tb2 in range(XC // 128):
                        tb = tq * (XC // 128) + tb2
                        for half in range(2):
                            h0 = half * 260
                            pvm = ps_mm.tile([128, 260], F32, tag="mm")
                            for k in range(KT):
                                nc.tensor.matmul(
                                    pvm[:], lhsT=xc[:, k, tb2 * 128:(tb2 + 1) * 128],
                                    rhs=wva_t[:, k, h0:h0 + 260],
                                    start=(k == 0), stop=False)
                            nc.tensor.matmul(pvm[:], lhsT=ones_t[:],
                                             rhs=bva_t[:, h0:h0 + 260],
                                             start=False, stop=True)
                            nc.scalar.copy(V_t[:, tb, h0:h0 + 260], pvm[:])

                # ---- qk-proj + rope for Q (m=0) and K (m=1) chunks ----
                for mloc, dest in ((0, qp_ts), (1, kp_ts)):
                    msel = 2 * p + mloc
                    mmp = ps_mm.tile([128, XC], F32, tag="mm")
                    for k in range(KT):
                        nc.tensor.matmul(
                            mmp[:], lhsT=wqk_pair[:, k, mloc * 128:(mloc + 1) * 128],
                            rhs=xc[:, k, :], start=(k == 0), stop=(k == KT - 1))
                    bcol = bqk_t[:, msel:msel + 1]
                    # T1 = (psum + b) * cos  (DVE, SBUF out)
                    t1 = t1p.tile([128, XC], F32, tag="t1")
                    nc.vector.scalar_tensor_tensor(
                        t1[:], mmp[:], bcol, cos_t[:, c0:c0 + XC],
                        op0=ALU.add, op1=ALU.mult)
                    # T2 = (psum + b) * sin  (DVE, PSUM out)
                    t2 = ps_s.tile([128, XC], F32, tag="s")
                    nc.vector.scalar_tensor_tensor(
                        t2[:], mmp[:], bcol, sin_t[:, c0:c0 + XC],
                        op0=ALU.add, op1=ALU.mult)
                    dsl = dest[c0 // QC][:, 0:XC]
                    for hh in range(2):
                        b0 = 64 * hh
                        # lo = e*c - o*s ; hi = e*s + o*c
                        nc.vector.tensor_sub(dsl[b0:b0 + 32, :],
                                             t1[b0:b0 + 32, :], t2[b0 + 32:b0 + 64, :])
                        nc.vector.tensor_add(dsl[b0 + 32:b0 + 64, :],
                                             t2[b0:b0 + 32, :], t1[b0 + 32:b0 + 64, :])

            # -------- attention for this pair --------
            at_qs = [atp.tile([128, QC], F32R, tag="attnT", name=f"at{p}_{i}") for i in range(NQC)]
            at_tiles.append(at_qs)
            for qc in range(NQC):
                q0c = qc * QC
                nkb = 4 * qc + 4
                pvA = ps_pv.tile([65, QC], F32, tag="pv")
                pvB = ps_pv.tile([65, QC], F32, tag="pv")
                s_tiles = {}

                def emit_s(kb):
                    d = kb - 4 * qc
                    v0 = 0 if d < 0 else min(128 * d, QC - 256)
                    sAB = ps_s.tile([128, 2, QC], F32, tag="s")
                    kq = kp_ts[kb // 4]
                    kc0 = (kb % 4) * 128
                    qq = qp_ts[qc]
                    nc.tensor.matmul(sAB[:, 0, v0:], lhsT=kq[0:64, kc0:kc0 + 128],
                                     rhs=qq[0:64, v0:],
                                     start=True, stop=True, tile_position=(0, 0))
                    nc.tensor.matmul(sAB[:, 1, v0:], lhsT=kq[64:128, kc0:kc0 + 128],
                                     rhs=qq[64:128, v0:],
                                     start=True, stop=True, tile_position=(64, 0))
                    s_tiles[kb] = (sAB, d, v0)

                emit_s(0)
                for kb in range(nkb):
                    if kb + 1 < nkb:
                        emit_s(kb + 1)
                    sAB, d, v0 = s_tiles.pop(kb)
                    if d == 3:
                        mb = bass.AP(mask3_t.tensor, mask3_t[:].offset,
                                     [mask3_t[:].ap[0], [0, 2], [1, 256]])
                        nc.vector.tensor_add(sAB[:, :, 256:512], sAB[:, :, 256:512], mb)
                    elif d >= 0:
                        mb = bass.AP(mask_t.tensor, mask_t[:].offset,
                                     [mask_t[:].ap[0], [0, 2], [1, 128]])
                        nc.vector.tensor_add(sAB[:, :, v0:v0 + 128],
                                             sAB[:, :, v0:v0 + 128], mb)
                    for hh, pv in ((0, pvA), (1, pvB)):
                        e = ep.tile([128, QC], F32R, tag="e")
                        nc.scalar.activation(e[:, v0:], sAB[:, hh, v0:], AF.Exp, scale=0.125)
                        nc.tensor.matmul(pv[0:65, v0:],
                                         lhsT=V_t[:, kb, (2 * p + hh) * 65:(2 * p + hh) * 65 + 65],
                                         rhs=e[:, v0:], start=(kb == 0), stop=(kb == nkb - 1))
                for hh, pv in ((0, pvA), (1, pvB)):
                    nc.vector.tensor_copy(at_qs[qc][64 * hh:64 * hh + 64, :], pv[0:64, :])
                    zrow = zbp.tile([1, QC], F32, tag="zrow")
                    nc.vector.tensor_copy(zrow[:], pv[64:65, :])
                    zb = zbp.tile([128, QC], F32, tag="zb")
                    nc.gpsimd.partition_broadcast(zb[:], zrow[:])
                    rz = zbp.tile([128, QC], F32, tag="rz")
                    nc.vector.reciprocal_approx_fast(rz[:], zb[:])
                    sl = at_qs[qc][64 * hh:64 * hh + 64, :]
                    nc.vector.tensor_mul(sl, sl, rz[64 * hh:64 * hh + 64, :])

        # -------- output projection --------
        qkv_ctx.close()
        wop = ctx.enter_context(tc.tile_pool(name="wop", bufs=1))
        outp = ctx.enter_context(tc.tile_pool(name="outp", bufs=3))
        wo_t = wop.tile([128, 4, 1024], F32R, tag="wo")
        nc.sync.dma_start(out=wo_t[:], in_=wo_d.rearrange("(k p) m -> p k m", p=128))
        for qb in range(16):
            for oc in range(2):
                po = ps_mm.tile([128, 512], F32, tag="mm")
                for p4 in range(NPAIR):
                    nc.tensor.matmul(
                        po[:], lhsT=at_tiles[p4][qb // 4][:, (qb % 4) * 128:(qb % 4) * 128 + 128],
                        rhs=wo_t[:, p4, oc * 512:(oc + 1) * 512],
                        start=(p4 == 0), stop=(p4 == NPAIR - 1))
                ot = outp.tile([128, 512], F32, tag="ot")
                nc.scalar.copy(ot[:], po[:])
                nc.sync.dma_start(out=out_d[qb * 128:(qb + 1) * 128,
                                            oc * 512:(oc + 1) * 512], in_=ot[:])


def _prep_inputs(x, W_qkv, b_qkv, W_out, cos, sin):
    """Host-side sharding/permutation. Returns list of 8 per-core in_maps."""
    x = np.ascontiguousarray(np.asarray(x, dtype=np.float32))
    W_qkv = np.asarray(W_qkv, dtype=np.float32)
    b_qkv = np.asarray(b_qkv, dtype=np.float32)
    W_out = np.asarray(W_out, dtype=np.float32)
    cos = np.asarray(cos, dtype=np.float32)
    sin = np.asarray(sin, dtype=np.float32)

    xTs = [np.ascontiguousarray(x[b].T) for b in range(B)]
    # rope tables: rows r = table[:, r % 32]
    cosT = np.ascontiguousarray(cos.T)           # [32, T]
    sinT = np.ascontiguousarray(sin.T)
    cos4 = np.ascontiguousarray(np.tile(cosT, (4, 1)))   # [128, T]
    sin4 = np.ascontiguousarray(np.tile(sinT, (4, 1)))
    ones1 = np.ones((1, 128), np.float32)

    groups = []
    for g in range(2):
        heads = [g * HPG + i for i in range(HPG)]
        qk_cols = []
        for p in range(NPAIR):
            A, Bh = heads[2 * p], heads[2 * p + 1]
            for base in (0, DK):                  # q block then k block
                for h in (A, Bh):
                    qk_cols += list(3 * DK * h + base + np.arange(0, DK, 2))
                    qk_cols += list(3 * DK * h + base + np.arange(1, DK, 2))
        qk_cols = np.array(qk_cols)
        wqk = np.ascontiguousarray(W_qkv[:, qk_cols])         # [1024, 1024]
        bqk = np.ascontiguousarray(b_qkv[qk_cols].reshape(8, 128).T)  # [128, 8]
        # v with interleaved zero cols at the ones positions: [1024, 8*65]
        wva = np.zeros((D, 520), np.float32)
        bva = np.zeros((1, 520), np.float32)
        for i, h in enumerate(heads):
            vcols = 3 * DK * h + 2 * DK + np.arange(DK)
            wva[:, i * 65:i * 65 + 64] = W_qkv[:, vcols]
            bva[0, i * 65:i * 65 + 64] = b_qkv[vcols]
            bva[0, i * 65 + 64] = 1.0                 # ones column
        wo = np.ascontiguousarray(W_out[g * 512:(g + 1) * 512, :])
        groups.append(dict(wqk=wqk, bqk=bqk, wva=np.ascontiguousarray(wva),
                           bva=bva, wo=wo))

    in_maps = []
    for c in range(NC_):
        b, g = c // 2, c % 2
        gr = groups[g]
        in_maps.append({
            "xT": xTs[b], "wqk": gr["wqk"], "wva": gr["wva"], "bva": gr["bva"],
            "ones1": ones1, "wo": gr["wo"], "bqk": gr["bqk"],
            "cos4": cos4, "sin4": sin4,
        })
    return in_maps


def run(x, W_qkv, b_qkv, W_out, b_out, cos, sin, trace=False, trace_cores=None):
    """Build/compile (cached), run on 8 cores, return (out, BassKernelResults)."""
    if "nc" not in _cache:
        _cache["nc"] = _build_nc()
    nc = _cache["nc"]
    in_maps = _prep_inputs(x, W_qkv, b_qkv, W_out, cos, sin)
    kw = {}
    if trace:
        kw = dict(trace=True, trace_cores=trace_cores or [0])
    res = bass_utils.run_bass_kernel_spmd(nc, in_maps, core_ids=list(range(NC_)), **kw)
    b_out = np.asarray(b_out, dtype=np.float32)
    out = np.empty((B, T, D), np.float32)
    for b in range(B):
        out[b] = res.results[2 * b]["out"] + res.results[2 * b + 1]["out"] + b_out[None, :]
    return out, res


def kernel(x, W_qkv, b_qkv, W_out, b_out, cos, sin):
    out, _ = run(x, W_qkv, b_qkv, W_out, b_out, cos, sin)
    return out



# revision 11
# speedup vs baseline: 1.2752x; 1.2752x over previous
"""Causal self-attention (B=4, T=2048, D=1024, H=16) on 8 TRN2 NeuronCores.

Sharding: core c handles batch b=c//2 and head-group g=c%2 (8 heads).
Each core computes its heads' attention + a partial output projection
(contraction over its 512 attn channels); the host sums the two partials
per batch and adds b_out.

v2: all matmul operands bf16 (psum stays fp32), and the emission order
interleaves pair p's attention blocks with pair p+1's QKV projection
(and, for the last pair, the output projection) so the PE instruction
stream never starves — keeping the tensor engine at the full 2.4 GHz
p-state instead of dropping to the 1.2 GHz mid state on every exp wait.

Per-core device pipeline (per head-pair p, heads packed 2/128-partitions):
  qk-proj   qkT[ch,T] = wqk.T @ xT  (bf16, ch-major), rope via DVE
            (stt psum->bf16, then 2x-mode bf16 sub/add)
  v-proj    V[t, 2*65] = xT.T @ wv_aug  (65th col per head = ones -> Z)
  S^T       [k,q] = k'^T q' per head, 2 heads in PE quadrants (K=64)
  softmax   exp((S+mask)*0.125) in ONE ACT instr per block (both heads),
            bf16 out; normalizer Z from the V ones col
  PV        attn_aug^T[65,q] = V_aug^T @ E^T accumulated over k blocks
  norm      rz = 1/Z (DVE recip from psum), partition-bcast (Pool),
            attnT = pv * rz -> bf16
  out-proj  out[q,o] = attnT.T @ wo  (partial; host adds pair partials)
"""
import sys
from collections import deque
import numpy as np

for _p in ("/opt/trn_rl_repo", "/root/.axon_site/_ro/trn_rl_repo"):
    if _p not in sys.path:
        sys.path.append(_p)

import ml_dtypes
import concourse.bass as bass
import concourse.bacc as bacc
import concourse.tile as tile
import concourse.mybir as mybir
from concourse import bass_utils

F32 = mybir.dt.float32
BF16 = mybir.dt.bfloat16
AF = mybir.ActivationFunctionType
ALU = mybir.AluOpType

B, T, D, H, DK = 4, 2048, 1024, 16, 64
NC_ = 8          # cores
HPG = 8          # heads per group
NPAIR = 4        # head pairs per core
KT = 8           # 128-row k-tiles over D
XC = 512         # x/qkv t-chunk width
NXC = T // XC    # 4
QC = 512         # attention q-chunk width
NQC = T // QC    # 4
NKB = T // 128   # 16 key blocks
MASK_VAL = -30000.0

_cache = {}
DEBUG = False


def _build_nc(trace_scopes=False):
    nc = bacc.Bacc("TRN2", target_bir_lowering=False, debug=False)

    xT_d = nc.dram_tensor("xT", [D, T], BF16, kind="ExternalInput").ap()
    wqk_d = nc.dram_tensor("wqk", [D, 1024], BF16, kind="ExternalInput").ap()
    wva_d = nc.dram_tensor("wva", [D, 520], BF16, kind="ExternalInput").ap()
    bva_d = nc.dram_tensor("bva", [1, 520], BF16, kind="ExternalInput").ap()
    ones_d = nc.dram_tensor("ones1", [1, 128], BF16, kind="ExternalInput").ap()
    wo_d = nc.dram_tensor("wo", [512, 1024], BF16, kind="ExternalInput").ap()
    bqk_d = nc.dram_tensor("bqk", [128, 8], F32, kind="ExternalInput").ap()
    cos_d = nc.dram_tensor("cos4", [128, T], F32, kind="ExternalInput").ap()
    sin_d = nc.dram_tensor("sin4", [128, T], F32, kind="ExternalInput").ap()
    out_d = nc.dram_tensor("out", [T, 1024], F32, kind="ExternalOutput").ap()
    dbg = {}
    if DEBUG:
        dbg["e"] = nc.dram_tensor("dbg_e", [128, 1024], BF16, kind="ExternalOutput").ap()
        dbg["zb"] = nc.dram_tensor("dbg_zb", [128, 512], F32, kind="ExternalOutput").ap()
        dbg["at"] = nc.dram_tensor("dbg_at", [128, 512], BF16, kind="ExternalOutput").ap()

    with tile.TileContext(nc, pool_alloc_mode="queue") as tc:
        _emit(tc, nc, xT_d, wqk_d, wva_d, bva_d, ones_d, wo_d, bqk_d,
              cos_d, sin_d, out_d, dbg)
    nc.compile()
    return nc


def _emit(tc, nc, xT_d, wqk_d, wva_d, bva_d, ones_d, wo_d, bqk_d,
          cos_d, sin_d, out_d, dbg=None):
    from contextlib import ExitStack
    ctx = ExitStack()
    with ctx:
        consts = ctx.enter_context(tc.tile_pool(name="consts", bufs=1))
        vpool = ctx.enter_context(tc.tile_pool(name="vpool", bufs=1))
        qkp = ctx.enter_context(tc.tile_pool(name="qkp", bufs=4))
        ep = ctx.enter_context(tc.tile_pool(name="ep", bufs=4))
        zbp = ctx.enter_context(tc.tile_pool(name="zbp", bufs=2))
        atp = ctx.enter_context(tc.tile_pool(name="atp", bufs=16))
        wqkp = ctx.enter_context(tc.tile_pool(name="wqkp", bufs=2))
        xp = ctx.enter_context(tc.tile_pool(name="xp", bufs=2))
        t1p = ctx.enter_context(tc.tile_pool(name="t1p", bufs=3))
        wop = ctx.enter_context(tc.tile_pool(name="wop", bufs=1))
        outp = ctx.enter_context(tc.tile_pool(name="outp", bufs=3))
        ps_mm = ctx.enter_context(tc.tile_pool(name="ps_mm", bufs=2, space="PSUM"))
        ps_s = ctx.enter_context(tc.tile_pool(name="ps_s", bufs=2, space="PSUM"))
        ps_pv = ctx.enter_context(tc.tile_pool(name="ps_pv", bufs=2, space="PSUM"))

        # ---------------- constants ----------------
        cos_t = consts.tile([128, T], F32, tag="cos")
        nc.sync.dma_start(out=cos_t[:], in_=cos_d)
        sin_t = consts.tile([128, T], F32, tag="sin")
        nc.sync.dma_start(out=sin_t[:], in_=sin_d)
        bqk_t = consts.tile([128, 8], F32, tag="bqk")
        nc.sync.dma_start(out=bqk_t[:], in_=bqk_d)
        bva_t = consts.tile([1, 520], BF16, tag="bva")
        nc.sync.dma_start(out=bva_t[:], in_=bva_d)
        ones_t = consts.tile([1, 128], BF16, tag="ones")
        nc.sync.dma_start(out=ones_t[:], in_=ones_d)
        wva_t = consts.tile([128, KT, 520], BF16, tag="wva")
        nc.sync.dma_start(out=wva_t[:], in_=wva_d.rearrange("(k p) m -> p k m", p=128))
        wo_t = wop.tile([128, 4, 1024], BF16, tag="wo")
        nc.sync.dma_start(out=wo_t[:], in_=wo_d.rearrange("(k p) m -> p k m", p=128))
        # additive causal mask block [128,128]: 0 where col>=row else MASK_VAL
        mask_t = consts.tile([128, 128], F32, tag="mask")
        nc.gpsimd.memset(mask_t[:], 0.0)
        nc.gpsimd.affine_select(
            out=mask_t[:], in_=mask_t[:], compare_op=ALU.is_ge, fill=MASK_VAL,
            base=0, pattern=[[1, 128]], channel_multiplier=-1)

        # per-pair V_aug tiles: [128 tok, 16 kb, 2 heads * 65]
        V_ts = [vpool.tile([128, NKB, 130], BF16, tag=f"V{p}", name=f"V{p}")
                for p in range(NPAIR)]

        xT_r = xT_d.rearrange("(k p) t -> p k t", p=128)
        wqk_r = wqk_d.rearrange("(k p) m -> p k m", p=128)

        # live per-pair state
        wqk_pair = [None] * NPAIR
        qp_ts = [None] * NPAIR
        kp_ts = [None] * NPAIR
        at_tiles = [[None] * NQC for _ in range(NPAIR)]
        xc_cur = [None]

        # ---------- QKV projection units (per pair, emission closures) ----
        def u_load_xc(p, tq):
            def f():
                c0 = tq * XC
                xc = xp.tile([128, KT, XC], BF16, tag="xc")
                nc.sync.dma_start(out=xc[:], in_=xT_r[:, :, c0:c0 + XC])
                xc_cur[0] = xc
            return f

        def u_qk_half(p, tq, mloc, half, mmp_box):
            # half 0: matmuls k=0..3 (alloc psum); half 1: k=4..7 + rope
            def f():
                c0 = tq * XC
                xc = xc_cur[0]
                if half == 0:
                    mmp_box[0] = ps_mm.tile([128, XC], F32, tag="mm", name="mmqk")
                mmp = mmp_box[0]
                for k in range(4 * half, 4 * half + 4):
                    nc.tensor.matmul(
                        mmp[:], lhsT=wqk_pair[p][:, k, mloc * 128:(mloc + 1) * 128],
                        rhs=xc[:, k, :], start=(k == 0), stop=(k == KT - 1))
                if half == 0:
                    return
                msel = 2 * p + mloc
                bcol = bqk_t[:, msel:msel + 1]
                dest = qp_ts[p] if mloc == 0 else kp_ts[p]
                # T1 = (psum + b) * cos ; T2 = (psum + b) * sin  (bf16 out)
                t1 = t1p.tile([128, XC], BF16, tag="t1")
                nc.vector.scalar_tensor_tensor(
                    t1[:], mmp[:], bcol, cos_t[:, c0:c0 + XC],
                    op0=ALU.add, op1=ALU.mult)
                # t2 lives in PSUM: cross-partition reads are exempt from the
                # SBUF same-start-partition rule the BIR verifier enforces
                t2 = ps_s.tile([128, XC], F32, tag="s", name="t2")
                nc.vector.scalar_tensor_tensor(
                    t2[:], mmp[:], bcol, sin_t[:, c0:c0 + XC],
                    op0=ALU.add, op1=ALU.mult)
                dsl = dest[c0 // QC][:, 0:XC]
                for hh in range(2):
                    b0 = 64 * hh
                    # lo = e*c - o*s ; hi = e*s + o*c
                    nc.vector.tensor_sub(dsl[b0:b0 + 32, :],
                                         t1[b0:b0 + 32, :], t2[b0 + 32:b0 + 64, :])
                    nc.vector.tensor_add(dsl[b0 + 32:b0 + 64, :],
                                         t2[b0:b0 + 32, :], t1[b0 + 32:b0 + 64, :])
            return f

        def u_v_tb(p, tq, tb2):
            def f():
                tb = tq * (XC // 128) + tb2
                xc = xc_cur[0]
                pvm = ps_mm.tile([128, 130], F32, tag="mm")
                h0 = 130 * p
                for k in range(KT):
                    nc.tensor.matmul(
                        pvm[:], lhsT=xc[:, k, tb2 * 128:(tb2 + 1) * 128],
                        rhs=wva_t[:, k, h0:h0 + 130],
                        start=(k == 0), stop=False)
                nc.tensor.matmul(pvm[:], lhsT=ones_t[:],
                                 rhs=bva_t[:, h0:h0 + 130],
                                 start=False, stop=True)
                nc.scalar.copy(V_ts[p][:, tb, :], pvm[:])
            return f

        def qkv_units(p):
            us = []
            us.append(lambda p=p: _alloc_pair(p))
            for tq in range(NXC):
                us.append(u_load_xc(p, tq))
                for mloc in range(2):
                    box = [None]
                    us.append(u_qk_half(p, tq, mloc, 0, box))
                    us.append(u_qk_half(p, tq, mloc, 1, box))
                for tb2 in range(XC // 128):
                    us.append(u_v_tb(p, tq, tb2))
            return us

        def _alloc_pair(p):
            wqk_pair[p] = wqkp.tile([128, KT, 256], BF16, tag="wqk",
                                    name=f"wqk{p}")
            nc.sync.dma_start(out=wqk_pair[p][:],
                              in_=wqk_r[:, :, 256 * p:256 * (p + 1)])
            qp_ts[p] = [qkp.tile([128, QC], BF16, tag=f"qp{p % 2}",
                                 name=f"qp{p}_{i}") for i in range(NQC)]
            kp_ts[p] = [qkp.tile([128, QC], BF16, tag=f"kp{p % 2}",
                                 name=f"kp{p}_{i}") for i in range(NQC)]

        # ---------- attention blocks (per pair, emission closures) --------
        def att_blocks(p):
            blocks = []
            for qc in range(NQC):
                st = {}
                blocks.append(_mk_qc_start(p, qc, st))
                nkb = 4 * qc + 4
                for kb in range(nkb):
                    blocks.append(_mk_block(p, qc, kb, st))
                blocks.append(_mk_qc_end(p, qc, st))
            return blocks

        def _emit_s(p, qc, kb, st):
            d = kb - 4 * qc
            v0 = 0 if d < 0 else min(128 * d, QC - 128)
            sAB = ps_s.tile([128, 2, QC], F32, tag="s")
            kq = kp_ts[p][kb // 4]
            kc0 = (kb % 4) * 128
            qq = qp_ts[p][qc]
            nc.tensor.matmul(sAB[:, 0, v0:], lhsT=kq[0:64, kc0:kc0 + 128],
                             rhs=qq[0:64, v0:],
                             start=True, stop=True, tile_position=(0, 0))
            nc.tensor.matmul(sAB[:, 1, v0:], lhsT=kq[64:128, kc0:kc0 + 128],
                             rhs=qq[64:128, v0:],
                             start=True, stop=True, tile_position=(64, 0))
            st[kb] = (sAB, d, v0)

        def _mk_qc_start(p, qc, st):
            def f():
                st["pvA"] = ps_pv.tile([65, QC], F32, tag="pv", name="pvA")
                st["pvB"] = ps_pv.tile([65, QC], F32, tag="pv", name="pvB")
                _emit_s(p, qc, 0, st)
            return f

        def _mk_block(p, qc, kb, st):
            nkb = 4 * qc + 4

            def f():
                if kb + 1 < nkb:
                    _emit_s(p, qc, kb + 1, st)
                sAB, d, v0 = st.pop(kb)
                if d >= 0:
                    mb = bass.AP(mask_t.tensor, mask_t[:].offset,
                                 [mask_t[:].ap[0], [0, 2], [1, 128]])
                    nc.vector.tensor_add(sAB[:, :, v0:v0 + 128],
                                         sAB[:, :, v0:v0 + 128], mb)
                # one ACT instr: exp of both heads -> bf16
                e = ep.tile([128, 2, QC], BF16, tag="e")
                nc.scalar.activation(e[:, :, v0:], sAB[:, :, v0:], AF.Exp,
                                     scale=0.125)
                if DEBUG and p == 0 and qc == 0 and kb == 0:
                    nc.sync.dma_start(out=dbg["e"], in_=e[:])
                for hh, pv in ((0, st["pvA"]), (1, st["pvB"])):
                    nc.tensor.matmul(pv[0:65, v0:],
                                     lhsT=V_ts[p][:, kb, 65 * hh:65 * hh + 65],
                                     rhs=e[:, hh, v0:],
                                     start=(kb == 0), stop=(kb == nkb - 1))
            return f

        def _mk_qc_end(p, qc, st):
            def f():
                at = atp.tile([128, QC], BF16, tag="attnT", name=f"at{p}_{qc}")
                at_tiles[p][qc] = at
                pvA, pvB = st["pvA"], st["pvB"]
                zrA = zbp.tile([1, QC], F32, tag="zrA", name="zrA")
                zrB = zbp.tile([1, QC], F32, tag="zrB", name="zrB")
                nc.vector.tensor_copy(zrA[:], pvA[64:65, :])
                nc.vector.tensor_copy(zrB[:], pvB[64:65, :])
                rzA = zbp.tile([1, QC], F32, tag="rzA", name="rzA")
                rzB = zbp.tile([1, QC], F32, tag="rzB", name="rzB")
                nc.vector.reciprocal_approx_fast(rzA[:], zrA[:])
                nc.vector.reciprocal_approx_fast(rzB[:], zrB[:])
                zbA = zbp.tile([128, QC], F32, tag="zbA", name="zbA")
                zbB = zbp.tile([128, QC], F32, tag="zbB", name="zbB")
                nc.gpsimd.partition_broadcast(zbA[:], rzA[:])
                nc.gpsimd.partition_broadcast(zbB[:], rzB[:])
                nc.vector.tensor_mul(at[0:64, :], pvA[0:64, :], zbA[0:64, :])
                nc.vector.tensor_mul(at[64:128, :], pvB[0:64, :], zbB[64:128, :])
                if DEBUG and p == 0 and qc == 0:
                    nc.sync.dma_start(out=dbg["zb"], in_=zbA[:])
                    nc.sync.dma_start(out=dbg["at"], in_=at[:])
                if p == NPAIR - 1:
                    for qb in range(4 * qc, 4 * qc + 4):
                        for oc in range(2):
                            fills.append(u_out(qb, oc))
            return f

        # ---------- output projection units ----------
        def u_out(qb, oc):
            def f():
                po = ps_mm.tile([128, 512], F32, tag="mm")
                for p4 in range(NPAIR):
                    nc.tensor.matmul(
                        po[:],
                        lhsT=at_tiles[p4][qb // 4][:, (qb % 4) * 128:(qb % 4) * 128 + 128],
                        rhs=wo_t[:, p4, oc * 512:(oc + 1) * 512],
                        start=(p4 == 0), stop=(p4 == NPAIR - 1))
                ot = outp.tile([128, 512], F32, tag="ot")
                nc.scalar.copy(ot[:], po[:])
                nc.sync.dma_start(out=out_d[qb * 128:(qb + 1) * 128,
                                            oc * 512:(oc + 1) * 512], in_=ot[:])
            return f

        # ---------- interleaved emission ----------
        fills = deque()
        for u in qkv_units(0):
            u()
        for p in range(NPAIR):
            if p + 1 < NPAIR:
                fills.extend(qkv_units(p + 1))
            blocks = att_blocks(p)
            for i, blk in enumerate(blocks):
                blk()
                if fills and (i % 5) != 4:
                    fills.popleft()()
            if p + 1 < NPAIR:
                # next pair's attention depends on ALL its QKV work
                while fills:
                    fills.popleft()()
        while fills:
            fills.popleft()()


def _prep_inputs(x, W_qkv, b_qkv, W_out, cos, sin):
    """Host-side sharding/permutation. Returns list of 8 per-core in_maps."""
    BF = ml_dtypes.bfloat16
    x = np.ascontiguousarray(np.asarray(x, dtype=np.float32))
    W_qkv = np.asarray(W_qkv, dtype=np.float32)
    b_qkv = np.asarray(b_qkv, dtype=np.float32)
    W_out = np.asarray(W_out, dtype=np.float32)
    cos = np.asarray(cos, dtype=np.float32)
    sin = np.asarray(sin, dtype=np.float32)

    xTs = [np.ascontiguousarray(x[b].T.astype(BF)) for b in range(B)]
    # rope tables: rows r = table[:, r % 32]
    cosT = np.ascontiguousarray(cos.T)           # [32, T]
    sinT = np.ascontiguousarray(sin.T)
    cos4 = np.ascontiguousarray(np.tile(cosT, (4, 1)))   # [128, T]
    sin4 = np.ascontiguousarray(np.tile(sinT, (4, 1)))
    ones1 = np.ones((1, 128), BF)

    groups = []
    for g in range(2):
        heads = [g * HPG + i for i in range(HPG)]
        qk_cols = []
        for p in range(NPAIR):
            A, Bh = heads[2 * p], heads[2 * p + 1]
            for base in (0, DK):                  # q block then k block
                for h in (A, Bh):
                    qk_cols += list(3 * DK * h + base + np.arange(0, DK, 2))
                    qk_cols += list(3 * DK * h + base + np.arange(1, DK, 2))
        qk_cols = np.array(qk_cols)
        wqk = np.ascontiguousarray(W_qkv[:, qk_cols].astype(BF))      # [1024, 1024]
        bqk = np.ascontiguousarray(b_qkv[qk_cols].reshape(8, 128).T)  # [128, 8]
        # v with interleaved ones cols, pair-major: [1024, 8*65]
        wva = np.zeros((D, 520), np.float32)
        bva = np.zeros((1, 520), np.float32)
        for i, h in enumerate(heads):
            vcols = 3 * DK * h + 2 * DK + np.arange(DK)
            wva[:, i * 65:i * 65 + 64] = W_qkv[:, vcols]
            bva[0, i * 65:i * 65 + 64] = b_qkv[vcols]
            bva[0, i * 65 + 64] = 1.0                 # ones column
        wo = np.ascontiguousarray(W_out[g * 512:(g + 1) * 512, :].astype(BF))
        groups.append(dict(wqk=wqk, bqk=bqk,
                           wva=np.ascontiguousarray(wva.astype(BF)),
                           bva=np.ascontiguousarray(bva.astype(BF)), wo=wo))

    in_maps = []
    for c in range(NC_):
        b, g = c // 2, c % 2
        gr = groups[g]
        in_maps.append({
            "xT": xTs[b], "wqk": gr["wqk"], "wva": gr["wva"], "bva": gr["bva"],
            "ones1": ones1, "wo": gr["wo"], "bqk": gr["bqk"],
            "cos4": cos4, "sin4": sin4,
        })
    return in_maps


def run(x, W_qkv, b_qkv, W_out, b_out, cos, sin, trace=False, trace_cores=None):
    """Build/compile (cached), run on 8 cores, return (out, BassKernelResults)."""
    if "nc" not in _cache:
        _cache["nc"] = _build_nc()
    nc = _cache["nc"]
    in_maps = _prep_inputs(x, W_qkv, b_qkv, W_out, cos, sin)
    kw = {}
    if trace:
        kw = dict(trace=True, trace_cores=trace_cores or [0])
    res = bass_utils.run_bass_kernel_spmd(nc, in_maps, core_ids=list(range(NC_)), **kw)
    b_out = np.asarray(b_out, dtype=np.float32)
    out = np.empty((B, T, D), np.float32)
    for b in range(B):
        out[b] = res.results[2 * b]["out"] + res.results[2 * b + 1]["out"] + b_out[None, :]
    return out, res


def kernel(x, W_qkv, b_qkv, W_out, b_out, cos, sin):
    out, _ = run(x, W_qkv, b_qkv, W_out, b_out, cos, sin)
    return out


# revision 13
# speedup vs baseline: 1.3444x; 1.0543x over previous
"""Causal self-attention (B=4, T=2048, D=1024, H=16) on 8 TRN2 NeuronCores.

Sharding: core c handles batch b=c//2 and head-group g=c%2 (8 heads).
Each core computes its heads' attention + a partial output projection
(contraction over its 512 attn channels); the host sums the two partials
per batch and adds b_out.

v2: all matmul operands bf16 (psum stays fp32), and the emission order
interleaves pair p's attention blocks with pair p+1's QKV projection
(and, for the last pair, the output projection) so the PE instruction
stream never starves — keeping the tensor engine at the full 2.4 GHz
p-state instead of dropping to the 1.2 GHz mid state on every exp wait.

Per-core device pipeline (per head-pair p, heads packed 2/128-partitions):
  qk-proj   qkT[ch,T] = wqk.T @ xT  (bf16, ch-major), rope via DVE
            (stt psum->bf16, then 2x-mode bf16 sub/add)
  v-proj    V[t, 2*65] = xT.T @ wv_aug  (65th col per head = ones -> Z)
  S^T       [k,q] = k'^T q' per head, 2 heads in PE quadrants (K=64)
  softmax   exp((S+mask)*0.125) in ONE ACT instr per block (both heads),
            bf16 out; normalizer Z from the V ones col
  PV        attn_aug^T[65,q] = V_aug^T @ E^T accumulated over k blocks
  norm      rz = 1/Z (DVE recip from psum), partition-bcast (Pool),
            attnT = pv * rz -> bf16
  out-proj  out[q,o] = attnT.T @ wo  (partial; host adds pair partials)
"""
import sys
from collections import deque
import numpy as np

for _p in ("/opt/trn_rl_repo", "/root/.axon_site/_ro/trn_rl_repo"):
    if _p not in sys.path:
        sys.path.append(_p)

import ml_dtypes
import concourse.bass as bass
import concourse.bacc as bacc
import concourse.tile as tile
import concourse.mybir as mybir
from concourse import bass_utils

F32 = mybir.dt.float32
BF16 = mybir.dt.bfloat16
AF = mybir.ActivationFunctionType
ALU = mybir.AluOpType

B, T, D, H, DK = 4, 2048, 1024, 16, 64
NC_ = 8          # cores
HPG = 8          # heads per group
NPAIR = 4        # head pairs per core
KT = 8           # 128-row k-tiles over D
XC = 512         # x/qkv t-chunk width
NXC = T // XC    # 4
QC = 512         # attention q-chunk width
NQC = T // QC    # 4
NKB = T // 128   # 16 key blocks
MASK_VAL = -30000.0

_cache = {}
DEBUG = False


def _build_nc(trace_scopes=False):
    nc = bacc.Bacc("TRN2", target_bir_lowering=False, debug=False)

    xT_d = nc.dram_tensor("xT", [D, T], BF16, kind="ExternalInput").ap()
    wqk_d = nc.dram_tensor("wqk", [D, 1024], BF16, kind="ExternalInput").ap()
    wva_d = nc.dram_tensor("wva", [D, 520], BF16, kind="ExternalInput").ap()
    bva_d = nc.dram_tensor("bva", [1, 520], BF16, kind="ExternalInput").ap()
    ones_d = nc.dram_tensor("ones1", [1, 128], BF16, kind="ExternalInput").ap()
    wo_d = nc.dram_tensor("wo", [512, 1024], BF16, kind="ExternalInput").ap()
    bqk_d = nc.dram_tensor("bqk", [128, 8], F32, kind="ExternalInput").ap()
    cos_d = nc.dram_tensor("cos4", [128, T], F32, kind="ExternalInput").ap()
    sin_d = nc.dram_tensor("sin4", [128, T], F32, kind="ExternalInput").ap()
    out_d = nc.dram_tensor("out", [T, 1024], F32, kind="ExternalOutput").ap()
    dbg = {}
    if DEBUG:
        dbg["e"] = nc.dram_tensor("dbg_e", [128, 1024], BF16, kind="ExternalOutput").ap()
        dbg["zb"] = nc.dram_tensor("dbg_zb", [128, 512], F32, kind="ExternalOutput").ap()
        dbg["at"] = nc.dram_tensor("dbg_at", [128, 512], BF16, kind="ExternalOutput").ap()

    with tile.TileContext(nc, pool_alloc_mode="queue") as tc:
        _emit(tc, nc, xT_d, wqk_d, wva_d, bva_d, ones_d, wo_d, bqk_d,
              cos_d, sin_d, out_d, dbg)
    nc.compile()
    return nc


def _emit(tc, nc, xT_d, wqk_d, wva_d, bva_d, ones_d, wo_d, bqk_d,
          cos_d, sin_d, out_d, dbg=None):
    from contextlib import ExitStack
    ctx = ExitStack()
    with ctx:
        consts = ctx.enter_context(tc.tile_pool(name="consts", bufs=1))
        vpool = ctx.enter_context(tc.tile_pool(name="vpool", bufs=1))
        qkp = ctx.enter_context(tc.tile_pool(name="qkp", bufs=4))
        ep = ctx.enter_context(tc.tile_pool(name="ep", bufs=4))
        zbp = ctx.enter_context(tc.tile_pool(name="zbp", bufs=2))
        atp = ctx.enter_context(tc.tile_pool(name="atp", bufs=16))
        wqkp = ctx.enter_context(tc.tile_pool(name="wqkp", bufs=2))
        xp = ctx.enter_context(tc.tile_pool(name="xp", bufs=2))
        t1p = ctx.enter_context(tc.tile_pool(name="t1p", bufs=3))
        wop = ctx.enter_context(tc.tile_pool(name="wop", bufs=1))
        outp = ctx.enter_context(tc.tile_pool(name="outp", bufs=3))
        ps_mm = ctx.enter_context(tc.tile_pool(name="ps_mm", bufs=2, space="PSUM"))
        ps_s = ctx.enter_context(tc.tile_pool(name="ps_s", bufs=2, space="PSUM"))
        ps_pv = ctx.enter_context(tc.tile_pool(name="ps_pv", bufs=2, space="PSUM"))

        # ---------------- constants ----------------
        cos_t = consts.tile([128, T], F32, tag="cos")
        nc.sync.dma_start(out=cos_t[:], in_=cos_d)
        sin_t = consts.tile([128, T], F32, tag="sin")
        nc.sync.dma_start(out=sin_t[:], in_=sin_d)
        bqk_t = consts.tile([128, 8], F32, tag="bqk")
        nc.sync.dma_start(out=bqk_t[:], in_=bqk_d)
        bva_t = consts.tile([1, 520], BF16, tag="bva")
        nc.sync.dma_start(out=bva_t[:], in_=bva_d)
        ones_t = consts.tile([1, 128], BF16, tag="ones")
        nc.sync.dma_start(out=ones_t[:], in_=ones_d)
        wva_t = consts.tile([128, KT, 520], BF16, tag="wva")
        nc.sync.dma_start(out=wva_t[:], in_=wva_d.rearrange("(k p) m -> p k m", p=128))
        wo_t = wop.tile([128, 4, 1024], BF16, tag="wo")
        nc.sync.dma_start(out=wo_t[:], in_=wo_d.rearrange("(k p) m -> p k m", p=128))
        # additive causal mask block [128,128]: 0 where col>=row else MASK_VAL
        mask_t = consts.tile([128, 128], F32, tag="mask")
        nc.gpsimd.memset(mask_t[:], 0.0)
        nc.gpsimd.affine_select(
            out=mask_t[:], in_=mask_t[:], compare_op=ALU.is_ge, fill=MASK_VAL,
            base=0, pattern=[[1, 128]], channel_multiplier=-1)

        # per-pair V_aug tiles: [128 tok, 16 kb, 2 heads * 65]
        V_ts = [vpool.tile([128, NKB, 130], BF16, tag=f"V{p}", name=f"V{p}")
                for p in range(NPAIR)]

        xT_r = xT_d.rearrange("(k p) t -> p k t", p=128)
        wqk_r = wqk_d.rearrange("(k p) m -> p k m", p=128)

        # live per-pair state
        wqk_pair = [None] * NPAIR
        qp_ts = [None] * NPAIR
        kp_ts = [None] * NPAIR
        at_tiles = [[None] * NQC for _ in range(NPAIR)]
        xc_cur = [None]

        # ---------- QKV projection units (per pair, emission closures) ----
        def u_load_xc(p, tq):
            def f():
                c0 = tq * XC
                xc = xp.tile([128, KT, XC], BF16, tag="xc")
                nc.sync.dma_start(out=xc[:], in_=xT_r[:, :, c0:c0 + XC])
                xc_cur[0] = xc
            return f

        def u_qk_half(p, tq, mloc, half, mmp_box):
            # half 0: matmuls k=0..3 (alloc psum); half 1: k=4..7 + rope
            def f():
                c0 = tq * XC
                xc = xc_cur[0]
                if half == 0:
                    mmp_box[0] = ps_mm.tile([128, XC], F32, tag="mm", name="mmqk")
                mmp = mmp_box[0]
                for k in range(4 * half, 4 * half + 4):
                    nc.tensor.matmul(
                        mmp[:], lhsT=wqk_pair[p][:, k, mloc * 128:(mloc + 1) * 128],
                        rhs=xc[:, k, :], start=(k == 0), stop=(k == KT - 1))
                if half == 0:
                    return
                msel = 2 * p + mloc
                bcol = bqk_t[:, msel:msel + 1]
                dest = qp_ts[p] if mloc == 0 else kp_ts[p]
                # T1 = (psum + b) * cos ; T2 = (psum + b) * sin  (bf16 out)
                t1 = t1p.tile([128, XC], BF16, tag="t1")
                nc.vector.scalar_tensor_tensor(
                    t1[:], mmp[:], bcol, cos_t[:, c0:c0 + XC],
                    op0=ALU.add, op1=ALU.mult)
                # t2 lives in PSUM: cross-partition reads are exempt from the
                # SBUF same-start-partition rule the BIR verifier enforces
                t2 = ps_s.tile([128, XC], F32, tag="s", name="t2")
                nc.vector.scalar_tensor_tensor(
                    t2[:], mmp[:], bcol, sin_t[:, c0:c0 + XC],
                    op0=ALU.add, op1=ALU.mult)
                dsl = dest[c0 // QC][:, 0:XC]
                for hh in range(2):
                    b0 = 64 * hh
                    # lo = e*c - o*s ; hi = e*s + o*c
                    nc.vector.tensor_sub(dsl[b0:b0 + 32, :],
                                         t1[b0:b0 + 32, :], t2[b0 + 32:b0 + 64, :])
                    nc.vector.tensor_add(dsl[b0 + 32:b0 + 64, :],
                                         t2[b0:b0 + 32, :], t1[b0 + 32:b0 + 64, :])
            return f

        def u_v_tb(p, tq, tb2):
            def f():
                tb = tq * (XC // 128) + tb2
                xc = xc_cur[0]
                pvm = ps_mm.tile([128, 130], F32, tag="mm")
                h0 = 130 * p
                for k in range(KT):
                    nc.tensor.matmul(
                        pvm[:], lhsT=xc[:, k, tb2 * 128:(tb2 + 1) * 128],
                        rhs=wva_t[:, k, h0:h0 + 130],
                        start=(k == 0), stop=False)
                nc.tensor.matmul(pvm[:], lhsT=ones_t[:],
                                 rhs=bva_t[:, h0:h0 + 130],
                                 start=False, stop=True)
                nc.scalar.copy(V_ts[p][:, tb, :], pvm[:])
            return f

        def qkv_units(p):
            us = []
            us.append(lambda p=p: _alloc_pair(p))
            for tq in range(NXC):
                us.append(u_load_xc(p, tq))
                for mloc in range(2):
                    box = [None]
                    us.append(u_qk_half(p, tq, mloc, 0, box))
                    us.append(u_qk_half(p, tq, mloc, 1, box))
                for tb2 in range(XC // 128):
                    us.append(u_v_tb(p, tq, tb2))
            return us

        def _alloc_pair(p):
            wqk_pair[p] = wqkp.tile([128, KT, 256], BF16, tag="wqk",
                                    name=f"wqk{p}")
            nc.sync.dma_start(out=wqk_pair[p][:],
                              in_=wqk_r[:, :, 256 * p:256 * (p + 1)])
            qp_ts[p] = [qkp.tile([128, QC], BF16, tag=f"qp{p % 2}",
                                 name=f"qp{p}_{i}") for i in range(NQC)]
            kp_ts[p] = [qkp.tile([128, QC], BF16, tag=f"kp{p % 2}",
                                 name=f"kp{p}_{i}") for i in range(NQC)]

        # ---------- attention blocks (per pair, emission closures) --------
        def att_blocks(p):
            blocks = []
            for qc in range(NQC):
                st = {}
                blocks.append(_mk_qc_start(p, qc, st))
                nkb = 4 * qc + 4
                for kb in range(nkb):
                    blocks.append(_mk_block(p, qc, kb, st))
                blocks.append(_mk_qc_end(p, qc, st))
            return blocks

        def _emit_s(p, qc, kb, st):
            d = kb - 4 * qc
            v0 = 0 if d < 0 else min(128 * d, QC - 128)
            sAB = ps_s.tile([128, 2, QC], F32, tag="s")
            kq = kp_ts[p][kb // 4]
            kc0 = (kb % 4) * 128
            qq = qp_ts[p][qc]
            nc.tensor.matmul(sAB[:, 0, v0:], lhsT=kq[0:64, kc0:kc0 + 128],
                             rhs=qq[0:64, v0:],
                             start=True, stop=True, tile_position=(0, 0))
            nc.tensor.matmul(sAB[:, 1, v0:], lhsT=kq[64:128, kc0:kc0 + 128],
                             rhs=qq[64:128, v0:],
                             start=True, stop=True, tile_position=(64, 0))
            st[kb] = (sAB, d, v0)

        def _mk_qc_start(p, qc, st):
            def f():
                st["pvA"] = ps_pv.tile([65, QC], F32, tag="pv", name="pvA")
                st["pvB"] = ps_pv.tile([65, QC], F32, tag="pv", name="pvB")
                _emit_s(p, qc, 0, st)
            return f

        def _mk_block(p, qc, kb, st):
            nkb = 4 * qc + 4

            def f():
                if kb + 1 < nkb:
                    _emit_s(p, qc, kb + 1, st)
                sAB, d, v0 = st.pop(kb)
                # one ACT instr: exp of both heads -> bf16
                e = ep.tile([128, 2, QC], BF16, tag="e")
                nc.scalar.activation(e[:, :, v0:], sAB[:, :, v0:], AF.Exp,
                                     scale=0.125)
                if d >= 0:
                    # zero the not-yet-causal triangle of E on the Pool
                    # engine (scores are bounded so unmasked exp is finite)
                    nc.gpsimd.affine_select(
                        out=e[:, :, v0:v0 + 128], in_=e[:, :, v0:v0 + 128],
                        compare_op=ALU.is_ge, fill=0.0,
                        base=0, pattern=[[0, 2], [1, 128]],
                        channel_multiplier=-1)
                if DEBUG and p == 0 and qc == 0 and kb == 0:
                    nc.sync.dma_start(out=dbg["e"], in_=e[:])
                for hh, pv in ((0, st["pvA"]), (1, st["pvB"])):
                    nc.tensor.matmul(pv[0:65, v0:],
                                     lhsT=V_ts[p][:, kb, 65 * hh:65 * hh + 65],
                                     rhs=e[:, hh, v0:],
                                     start=(kb == 0), stop=(kb == nkb - 1))
            return f

        def _mk_qc_end(p, qc, st):
            def f():
                at = atp.tile([128, QC], BF16, tag="attnT", name=f"at{p}_{qc}")
                at_tiles[p][qc] = at
                pvA, pvB = st["pvA"], st["pvB"]
                zrA = zbp.tile([1, QC], F32, tag="zrA", name="zrA")
                zrB = zbp.tile([1, QC], F32, tag="zrB", name="zrB")
                nc.vector.tensor_copy(zrA[:], pvA[64:65, :])
                nc.vector.tensor_copy(zrB[:], pvB[64:65, :])
                rzA = zbp.tile([1, QC], F32, tag="rzA", name="rzA")
                rzB = zbp.tile([1, QC], F32, tag="rzB", name="rzB")
                nc.vector.reciprocal_approx_fast(rzA[:], zrA[:])
                nc.vector.reciprocal_approx_fast(rzB[:], zrB[:])
                zbA = zbp.tile([128, QC], F32, tag="zbA", name="zbA")
                zbB = zbp.tile([128, QC], F32, tag="zbB", name="zbB")
                nc.gpsimd.partition_broadcast(zbA[:], rzA[:])
                nc.gpsimd.partition_broadcast(zbB[:], rzB[:])
                nc.vector.tensor_mul(at[0:64, :], pvA[0:64, :], zbA[0:64, :])
                nc.vector.tensor_mul(at[64:128, :], pvB[0:64, :], zbB[64:128, :])
                if DEBUG and p == 0 and qc == 0:
                    nc.sync.dma_start(out=dbg["zb"], in_=zbA[:])
                    nc.sync.dma_start(out=dbg["at"], in_=at[:])
                if p == NPAIR - 1:
                    for qb in range(4 * qc, 4 * qc + 4):
                        for oc in range(2):
                            fills.append(u_out(qb, oc))
            return f

        # ---------- output projection units ----------
        def u_out(qb, oc):
            def f():
                po = ps_mm.tile([128, 512], F32, tag="mm")
                for p4 in range(NPAIR):
                    nc.tensor.matmul(
                        po[:],
                        lhsT=at_tiles[p4][qb // 4][:, (qb % 4) * 128:(qb % 4) * 128 + 128],
                        rhs=wo_t[:, p4, oc * 512:(oc + 1) * 512],
                        start=(p4 == 0), stop=(p4 == NPAIR - 1))
                ot = outp.tile([128, 512], F32, tag="ot")
                nc.scalar.copy(ot[:], po[:])
                nc.sync.dma_start(out=out_d[qb * 128:(qb + 1) * 128,
                                            oc * 512:(oc + 1) * 512], in_=ot[:])
            return f

        # ---------- interleaved emission ----------
        fills = deque()
        for u in qkv_units(0):
            u()
        for p in range(NPAIR):
            if p + 1 < NPAIR:
                fills.extend(qkv_units(p + 1))
            blocks = att_blocks(p)
            for i, blk in enumerate(blocks):
                blk()
                if fills and (i % 5) != 4:
                    fills.popleft()()
            if p + 1 < NPAIR:
                # next pair's attention depends on ALL its QKV work
                while fills:
                    fills.popleft()()
        while fills:
            fills.popleft()()


def _prep_inputs(x, W_qkv, b_qkv, W_out, cos, sin):
    """Host-side sharding/permutation. Returns list of 8 per-core in_maps."""
    BF = ml_dtypes.bfloat16
    x = np.ascontiguousarray(np.asarray(x, dtype=np.float32))
    W_qkv = np.asarray(W_qkv, dtype=np.float32)
    b_qkv = np.asarray(b_qkv, dtype=np.float32)
    W_out = np.asarray(W_out, dtype=np.float32)
    cos = np.asarray(cos, dtype=np.float32)
    sin = np.asarray(sin, dtype=np.float32)

    xTs = [np.ascontiguousarray(x[b].T.astype(BF)) for b in range(B)]
    # rope tables: rows r = table[:, r % 32]
    cosT = np.ascontiguousarray(cos.T)           # [32, T]
    sinT = np.ascontiguousarray(sin.T)
    cos4 = np.ascontiguousarray(np.tile(cosT, (4, 1)))   # [128, T]
    sin4 = np.ascontiguousarray(np.tile(sinT, (4, 1)))
    ones1 = np.ones((1, 128), BF)

    groups = []
    for g in range(2):
        heads = [g * HPG + i for i in range(HPG)]
        qk_cols = []
        for p in range(NPAIR):
            A, Bh = heads[2 * p], heads[2 * p + 1]
            for base in (0, DK):                  # q block then k block
                for h in (A, Bh):
                    qk_cols += list(3 * DK * h + base + np.arange(0, DK, 2))
                    qk_cols += list(3 * DK * h + base + np.arange(1, DK, 2))
        qk_cols = np.array(qk_cols)
        wqk = np.ascontiguousarray(W_qkv[:, qk_cols].astype(BF))      # [1024, 1024]
        bqk = np.ascontiguousarray(b_qkv[qk_cols].reshape(8, 128).T)  # [128, 8]
        # v with interleaved ones cols, pair-major: [1024, 8*65]
        wva = np.zeros((D, 520), np.float32)
        bva = np.zeros((1, 520), np.float32)
        for i, h in enumerate(heads):
            vcols = 3 * DK * h + 2 * DK + np.arange(DK)
            wva[:, i * 65:i * 65 + 64] = W_qkv[:, vcols]
            bva[0, i * 65:i * 65 + 64] = b_qkv[vcols]
            bva[0, i * 65 + 64] = 1.0                 # ones column
        wo = np.ascontiguousarray(W_out[g * 512:(g + 1) * 512, :].astype(BF))
        groups.append(dict(wqk=wqk, bqk=bqk,
                           wva=np.ascontiguousarray(wva.astype(BF)),
                           bva=np.ascontiguousarray(bva.astype(BF)), wo=wo))

    in_maps = []
    for c in range(NC_):
        b, g = c // 2, c % 2
        gr = groups[g]
        in_maps.append({
            "xT": xTs[b], "wqk": gr["wqk"], "wva": gr["wva"], "bva": gr["bva"],
            "ones1": ones1, "wo": gr["wo"], "bqk": gr["bqk"],
            "cos4": cos4, "sin4": sin4,
        })
    return in_maps


def run(x, W_qkv, b_qkv, W_out, b_out, cos, sin, trace=False, trace_cores=None):
    """Build/compile (cached), run on 8 cores, return (out, BassKernelResults)."""
    if "nc" not in _cache:
        _cache["nc"] = _build_nc()
    nc = _cache["nc"]
    in_maps = _prep_inputs(x, W_qkv, b_qkv, W_out, cos, sin)
    kw = {}
    if trace:
        kw = dict(trace=True, trace_cores=trace_cores or [0])
    res = bass_utils.run_bass_kernel_spmd(nc, in_maps, core_ids=list(range(NC_)), **kw)
    b_out = np.asarray(b_out, dtype=np.float32)
    out = np.empty((B, T, D), np.float32)
    for b in range(B):
        out[b] = res.results[2 * b]["out"] + res.results[2 * b + 1]["out"] + b_out[None, :]
    return out, res


def kernel(x, W_qkv, b_qkv, W_out, b_out, cos, sin):
    out, _ = run(x, W_qkv, b_qkv, W_out, b_out, cos, sin)
    return out


# revision 14
# speedup vs baseline: 1.4187x; 1.0553x over previous
"""Causal self-attention (B=4, T=2048, D=1024, H=16) on 8 TRN2 NeuronCores.

Sharding: core c handles batch b=c//2 and head-group g=c%2 (8 heads).
Each core computes its heads' attention + a partial output projection
(contraction over its 512 attn channels); the host sums the two partials
per batch and adds b_out.

v2: all matmul operands bf16 (psum stays fp32), and the emission order
interleaves pair p's attention blocks with pair p+1's QKV projection
(and, for the last pair, the output projection) so the PE instruction
stream never starves — keeping the tensor engine at the full 2.4 GHz
p-state instead of dropping to the 1.2 GHz mid state on every exp wait.

Per-core device pipeline (per head-pair p, heads packed 2/128-partitions):
  qk-proj   qkT[ch,T] = wqk.T @ xT  (bf16, ch-major), rope via DVE
            (stt psum->bf16, then 2x-mode bf16 sub/add)
  v-proj    V[t, 2*65] = xT.T @ wv_aug  (65th col per head = ones -> Z)
  S^T       [k,q] = k'^T q' per head, 2 heads in PE quadrants (K=64)
  softmax   exp((S+mask)*0.125) in ONE ACT instr per block (both heads),
            bf16 out; normalizer Z from the V ones col
  PV        attn_aug^T[65,q] = V_aug^T @ E^T accumulated over k blocks
  norm      rz = 1/Z (DVE recip from psum), partition-bcast (Pool),
            attnT = pv * rz -> bf16
  out-proj  out[q,o] = attnT.T @ wo  (partial; host adds pair partials)
"""
import sys
from collections import deque
import numpy as np

for _p in ("/opt/trn_rl_repo", "/root/.axon_site/_ro/trn_rl_repo"):
    if _p not in sys.path:
        sys.path.append(_p)

import ml_dtypes
import concourse.bass as bass
import concourse.bacc as bacc
import concourse.tile as tile
import concourse.mybir as mybir
from concourse import bass_utils

F32 = mybir.dt.float32
BF16 = mybir.dt.bfloat16
AF = mybir.ActivationFunctionType
ALU = mybir.AluOpType

B, T, D, H, DK = 4, 2048, 1024, 16, 64
NC_ = 8          # cores
HPG = 8          # heads per group
NPAIR = 4        # head pairs per core
KT = 8           # 128-row k-tiles over D
XC = 512         # x/qkv t-chunk width
NXC = T // XC    # 4
QC = 512         # attention q-chunk width
NQC = T // QC    # 4
NKB = T // 128   # 16 key blocks
MASK_VAL = -30000.0

_cache = {}
DEBUG = False


def _build_nc(trace_scopes=False):
    nc = bacc.Bacc("TRN2", target_bir_lowering=False, debug=False)

    xT_d = nc.dram_tensor("xT", [D, T], BF16, kind="ExternalInput").ap()
    wqk_d = nc.dram_tensor("wqk", [D, 1024], BF16, kind="ExternalInput").ap()
    wva_d = nc.dram_tensor("wva", [D, 520], BF16, kind="ExternalInput").ap()
    bva_d = nc.dram_tensor("bva", [1, 520], BF16, kind="ExternalInput").ap()
    ones_d = nc.dram_tensor("ones1", [1, 128], BF16, kind="ExternalInput").ap()
    wo_d = nc.dram_tensor("wo", [512, 1024], BF16, kind="ExternalInput").ap()
    bqk_d = nc.dram_tensor("bqk", [128, 8], F32, kind="ExternalInput").ap()
    cos_d = nc.dram_tensor("cos4", [128, T], F32, kind="ExternalInput").ap()
    sin_d = nc.dram_tensor("sin4", [128, T], F32, kind="ExternalInput").ap()
    out_d = nc.dram_tensor("out", [T, 1024], F32, kind="ExternalOutput").ap()
    dbg = {}
    if DEBUG:
        dbg["e"] = nc.dram_tensor("dbg_e", [128, 1024], BF16, kind="ExternalOutput").ap()
        dbg["zb"] = nc.dram_tensor("dbg_zb", [128, 512], F32, kind="ExternalOutput").ap()
        dbg["at"] = nc.dram_tensor("dbg_at", [128, 512], BF16, kind="ExternalOutput").ap()

    with tile.TileContext(nc, pool_alloc_mode="queue") as tc:
        _emit(tc, nc, xT_d, wqk_d, wva_d, bva_d, ones_d, wo_d, bqk_d,
              cos_d, sin_d, out_d, dbg)
    nc.compile()
    return nc


def _emit(tc, nc, xT_d, wqk_d, wva_d, bva_d, ones_d, wo_d, bqk_d,
          cos_d, sin_d, out_d, dbg=None):
    from contextlib import ExitStack
    ctx = ExitStack()
    with ctx:
        consts = ctx.enter_context(tc.tile_pool(name="consts", bufs=1))
        vpool = ctx.enter_context(tc.tile_pool(name="vpool", bufs=1))
        qkp = ctx.enter_context(tc.tile_pool(name="qkp", bufs=4))
        ep = ctx.enter_context(tc.tile_pool(name="ep", bufs=4))
        zbp = ctx.enter_context(tc.tile_pool(name="zbp", bufs=2))
        atp = ctx.enter_context(tc.tile_pool(name="atp", bufs=16))
        wqkp = ctx.enter_context(tc.tile_pool(name="wqkp", bufs=2))
        xp = ctx.enter_context(tc.tile_pool(name="xp", bufs=2))
        t1p = ctx.enter_context(tc.tile_pool(name="t1p", bufs=3))
        wop = ctx.enter_context(tc.tile_pool(name="wop", bufs=1))
        outp = ctx.enter_context(tc.tile_pool(name="outp", bufs=3))
        ps_mm = ctx.enter_context(tc.tile_pool(name="ps_mm", bufs=2, space="PSUM"))
        ps_s = ctx.enter_context(tc.tile_pool(name="ps_s", bufs=2, space="PSUM"))
        ps_pv = ctx.enter_context(tc.tile_pool(name="ps_pv", bufs=2, space="PSUM"))

        # ---------------- constants ----------------
        cos_t = consts.tile([128, T], F32, tag="cos")
        nc.sync.dma_start(out=cos_t[:], in_=cos_d)
        sin_t = consts.tile([128, T], F32, tag="sin")
        nc.sync.dma_start(out=sin_t[:], in_=sin_d)
        bqk_t = consts.tile([128, 8], F32, tag="bqk")
        nc.sync.dma_start(out=bqk_t[:], in_=bqk_d)
        bva_t = consts.tile([1, 520], BF16, tag="bva")
        nc.sync.dma_start(out=bva_t[:], in_=bva_d)
        ones_t = consts.tile([1, 128], BF16, tag="ones")
        nc.sync.dma_start(out=ones_t[:], in_=ones_d)
        wva_t = consts.tile([128, KT, 520], BF16, tag="wva")
        nc.sync.dma_start(out=wva_t[:], in_=wva_d.rearrange("(k p) m -> p k m", p=128))
        wo_t = wop.tile([128, 4, 1024], BF16, tag="wo")
        nc.sync.dma_start(out=wo_t[:], in_=wo_d.rearrange("(k p) m -> p k m", p=128))
        # additive causal mask block [128,128]: 0 where col>=row else MASK_VAL
        mask_t = consts.tile([128, 128], F32, tag="mask")
        nc.gpsimd.memset(mask_t[:], 0.0)
        nc.gpsimd.affine_select(
            out=mask_t[:], in_=mask_t[:], compare_op=ALU.is_ge, fill=MASK_VAL,
            base=0, pattern=[[1, 128]], channel_multiplier=-1)

        # V_aug tiles per pair-couple: [128 tok, 16 kb, 4 heads * 65]
        V2_ts = [vpool.tile([128, NKB, 260], BF16, tag=f"V{g}", name=f"V{g}")
                 for g in range(2)]

        xT_r = xT_d.rearrange("(k p) t -> p k t", p=128)
        wqk_r = wqk_d.rearrange("(k p) m -> p k m", p=128)

        # live per-pair state
        wqk_pair = [None] * NPAIR
        qp_ts = [None] * NPAIR
        kp_ts = [None] * NPAIR
        at_tiles = [[None] * NQC for _ in range(NPAIR)]
        xc_cur = [None]

        # ---------- QKV projection units (per pair, emission closures) ----
        def u_load_xc(p, tq):
            def f():
                c0 = tq * XC
                xc = xp.tile([128, KT, XC], BF16, tag="xc")
                nc.sync.dma_start(out=xc[:], in_=xT_r[:, :, c0:c0 + XC])
                xc_cur[0] = xc
            return f

        def u_qk_half(p, tq, mloc, half, mmp_box):
            # half 0: matmuls k=0..3 (alloc psum); half 1: k=4..7 + rope
            def f():
                c0 = tq * XC
                xc = xc_cur[0]
                if half == 0:
                    mmp_box[0] = ps_mm.tile([128, XC], F32, tag="mm", name="mmqk")
                mmp = mmp_box[0]
                for k in range(4 * half, 4 * half + 4):
                    nc.tensor.matmul(
                        mmp[:], lhsT=wqk_pair[p][:, k, mloc * 128:(mloc + 1) * 128],
                        rhs=xc[:, k, :], start=(k == 0), stop=(k == KT - 1))
                if half == 0:
                    return
                msel = 2 * p + mloc
                bcol = bqk_t[:, msel:msel + 1]
                dest = qp_ts[p] if mloc == 0 else kp_ts[p]
                # T1 = (psum + b) * cos ; T2 = (psum + b) * sin  (bf16 out)
                t1 = t1p.tile([128, XC], BF16, tag="t1")
                nc.vector.scalar_tensor_tensor(
                    t1[:], mmp[:], bcol, cos_t[:, c0:c0 + XC],
                    op0=ALU.add, op1=ALU.mult)
                # t2 lives in PSUM: cross-partition reads are exempt from the
                # SBUF same-start-partition rule the BIR verifier enforces
                t2 = ps_s.tile([128, XC], F32, tag="s", name="t2")
                nc.vector.scalar_tensor_tensor(
                    t2[:], mmp[:], bcol, sin_t[:, c0:c0 + XC],
                    op0=ALU.add, op1=ALU.mult)
                dsl = dest[c0 // QC][:, 0:XC]
                for hh in range(2):
                    b0 = 64 * hh
                    # lo = e*c - o*s ; hi = e*s + o*c
                    nc.vector.tensor_sub(dsl[b0:b0 + 32, :],
                                         t1[b0:b0 + 32, :], t2[b0 + 32:b0 + 64, :])
                    nc.vector.tensor_add(dsl[b0 + 32:b0 + 64, :],
                                         t2[b0:b0 + 32, :], t1[b0 + 32:b0 + 64, :])
            return f

        def u_v_tb(p, tq, tb2):
            # computes V_aug for the pair-couple (p, p+1); called for even p
            def f():
                tb = tq * (XC // 128) + tb2
                xc = xc_cur[0]
                pvm = ps_mm.tile([128, 260], F32, tag="mm")
                h0 = 260 * (p // 2)
                for k in range(KT):
                    nc.tensor.matmul(
                        pvm[:], lhsT=xc[:, k, tb2 * 128:(tb2 + 1) * 128],
                        rhs=wva_t[:, k, h0:h0 + 260],
                        start=(k == 0), stop=False)
                nc.tensor.matmul(pvm[:], lhsT=ones_t[:],
                                 rhs=bva_t[:, h0:h0 + 260],
                                 start=False, stop=True)
                nc.scalar.copy(V2_ts[p // 2][:, tb, :], pvm[:])
            return f

        def qkv_chunk_units(p, tq):
            us = []
            if tq == 0:
                us.append(lambda p=p: _alloc_pair(p))
            us.append(u_load_xc(p, tq))
            for mloc in range(2):
                box = [None]
                us.append(u_qk_half(p, tq, mloc, 0, box))
                us.append(u_qk_half(p, tq, mloc, 1, box))
            if p % 2 == 0:
                for tb2 in range(XC // 128):
                    us.append(u_v_tb(p, tq, tb2))
            return us

        def _alloc_pair(p):
            wqk_pair[p] = wqkp.tile([128, KT, 256], BF16, tag="wqk",
                                    name=f"wqk{p}")
            nc.sync.dma_start(out=wqk_pair[p][:],
                              in_=wqk_r[:, :, 256 * p:256 * (p + 1)])
            qp_ts[p] = [qkp.tile([128, QC], BF16, tag=f"qp{p % 2}",
                                 name=f"qp{p}_{i}") for i in range(NQC)]
            kp_ts[p] = [qkp.tile([128, QC], BF16, tag=f"kp{p % 2}",
                                 name=f"kp{p}_{i}") for i in range(NQC)]

        # ---------- attention blocks (per pair, emission closures) --------
        def att_blocks(p):
            blocks = []
            for qc in range(NQC):
                st = {}
                blocks.append((qc, _mk_qc_start(p, qc, st)))
                nkb = 4 * qc + 4
                for kb in range(nkb):
                    blocks.append((qc, _mk_block(p, qc, kb, st)))
                blocks.append((qc, _mk_qc_end(p, qc, st)))
            return blocks

        def _emit_s(p, qc, kb, st):
            d = kb - 4 * qc
            v0 = 0 if d < 0 else min(128 * d, QC - 128)
            sAB = ps_s.tile([128, 2, QC], F32, tag="s")
            kq = kp_ts[p][kb // 4]
            kc0 = (kb % 4) * 128
            qq = qp_ts[p][qc]
            nc.tensor.matmul(sAB[:, 0, v0:], lhsT=kq[0:64, kc0:kc0 + 128],
                             rhs=qq[0:64, v0:],
                             start=True, stop=True, tile_position=(0, 0))
            nc.tensor.matmul(sAB[:, 1, v0:], lhsT=kq[64:128, kc0:kc0 + 128],
                             rhs=qq[64:128, v0:],
                             start=True, stop=True, tile_position=(64, 0))
            st[kb] = (sAB, d, v0)

        def _mk_qc_start(p, qc, st):
            def f():
                st["pvA"] = ps_pv.tile([65, QC], F32, tag="pv", name="pvA")
                st["pvB"] = ps_pv.tile([65, QC], F32, tag="pv", name="pvB")
                _emit_s(p, qc, 0, st)
            return f

        def _mk_block(p, qc, kb, st):
            nkb = 4 * qc + 4

            def f():
                if kb + 1 < nkb:
                    _emit_s(p, qc, kb + 1, st)
                sAB, d, v0 = st.pop(kb)
                # one ACT instr: exp of both heads -> bf16
                e = ep.tile([128, 2, QC], BF16, tag="e")
                nc.scalar.activation(e[:, :, v0:], sAB[:, :, v0:], AF.Exp,
                                     scale=0.125)
                if d >= 0:
                    # zero the not-yet-causal triangle of E on the Pool
                    # engine (scores are bounded so unmasked exp is finite)
                    nc.gpsimd.affine_select(
                        out=e[:, :, v0:v0 + 128], in_=e[:, :, v0:v0 + 128],
                        compare_op=ALU.is_ge, fill=0.0,
                        base=0, pattern=[[0, 2], [1, 128]],
                        channel_multiplier=-1)
                if DEBUG and p == 0 and qc == 0 and kb == 0:
                    nc.sync.dma_start(out=dbg["e"], in_=e[:])
                for hh, pv in ((0, st["pvA"]), (1, st["pvB"])):
                    c0 = 130 * (p % 2) + 65 * hh
                    nc.tensor.matmul(pv[0:65, v0:],
                                     lhsT=V2_ts[p // 2][:, kb, c0:c0 + 65],
                                     rhs=e[:, hh, v0:],
                                     start=(kb == 0), stop=(kb == nkb - 1))
            return f

        def _mk_qc_end(p, qc, st):
            def f():
                at = atp.tile([128, QC], BF16, tag="attnT", name=f"at{p}_{qc}")
                at_tiles[p][qc] = at
                pvA, pvB = st["pvA"], st["pvB"]
                zrA = zbp.tile([1, QC], F32, tag="zrA", name="zrA")
                zrB = zbp.tile([1, QC], F32, tag="zrB", name="zrB")
                nc.vector.tensor_copy(zrA[:], pvA[64:65, :])
                nc.vector.tensor_copy(zrB[:], pvB[64:65, :])
                rzA = zbp.tile([1, QC], F32, tag="rzA", name="rzA")
                rzB = zbp.tile([1, QC], F32, tag="rzB", name="rzB")
                nc.vector.reciprocal_approx_fast(rzA[:], zrA[:])
                nc.vector.reciprocal_approx_fast(rzB[:], zrB[:])
                zbA = zbp.tile([128, QC], F32, tag="zbA", name="zbA")
                zbB = zbp.tile([128, QC], F32, tag="zbB", name="zbB")
                nc.gpsimd.partition_broadcast(zbA[:], rzA[:])
                nc.gpsimd.partition_broadcast(zbB[:], rzB[:])
                nc.vector.tensor_mul(at[0:64, :], pvA[0:64, :], zbA[0:64, :])
                nc.vector.tensor_mul(at[64:128, :], pvB[0:64, :], zbB[64:128, :])
                if DEBUG and p == 0 and qc == 0:
                    nc.sync.dma_start(out=dbg["zb"], in_=zbA[:])
                    nc.sync.dma_start(out=dbg["at"], in_=at[:])
                if p == NPAIR - 1:
                    for qb in range(4 * qc, 4 * qc + 4):
                        for oc in range(2):
                            fills.append((p, 99, u_out(qb, oc), False))
            return f

        # ---------- output projection units ----------
        def u_out(qb, oc):
            def f():
                po = ps_mm.tile([128, 512], F32, tag="mm")
                for p4 in range(NPAIR):
                    nc.tensor.matmul(
                        po[:],
                        lhsT=at_tiles[p4][qb // 4][:, (qb % 4) * 128:(qb % 4) * 128 + 128],
                        rhs=wo_t[:, p4, oc * 512:(oc + 1) * 512],
                        start=(p4 == 0), stop=(p4 == NPAIR - 1))
                ot = outp.tile([128, 512], F32, tag="ot")
                nc.scalar.copy(ot[:], po[:])
                nc.sync.dma_start(out=out_d[qb * 128:(qb + 1) * 128,
                                            oc * 512:(oc + 1) * 512], in_=ot[:])
            return f

        # ---------- unified gated pipeline emission ----------
        # fills = future QKV chunk units (+ final out-proj units), popped as
        # PE fill during attention; gates[p] = chunks of pair p emitted.
        # ATT(p) qc j only requires QKV(p) chunks 0..j (qp[j], kp[<=j], V tb
        # <= 4j+3), enforced by the gate flush before each block.
        fills = deque()
        gates = [0] * NPAIR

        def pump():
            p_, tq_, u, last = fills.popleft()
            u()
            if last:
                gates[p_] = tq_ + 1

        for u in qkv_chunk_units(0, 0):
            u()
        gates[0] = 1
        for p in range(NPAIR):
            for tq in range(1 if p == 0 else 0, NXC):
                us = qkv_chunk_units(p, tq)
                fills.extend((p, tq, u, i == len(us) - 1)
                             for i, u in enumerate(us))
        for p in range(NPAIR):
            for i, (qc, blk) in enumerate(att_blocks(p)):
                while gates[p] <= qc and fills:
                    pump()
                blk()
                if fills and (i % 5) != 4:
                    pump()
        while fills:
            pump()


def _prep_inputs(x, W_qkv, b_qkv, W_out, cos, sin):
    """Host-side sharding/permutation. Returns list of 8 per-core in_maps."""
    BF = ml_dtypes.bfloat16
    x = np.ascontiguousarray(np.asarray(x, dtype=np.float32))
    W_qkv = np.asarray(W_qkv, dtype=np.float32)
    b_qkv = np.asarray(b_qkv, dtype=np.float32)
    W_out = np.asarray(W_out, dtype=np.float32)
    cos = np.asarray(cos, dtype=np.float32)
    sin = np.asarray(sin, dtype=np.float32)

    xTs = [np.ascontiguousarray(x[b].T.astype(BF)) for b in range(B)]
    # rope tables: rows r = table[:, r % 32]
    cosT = np.ascontiguousarray(cos.T)           # [32, T]
    sinT = np.ascontiguousarray(sin.T)
    cos4 = np.ascontiguousarray(np.tile(cosT, (4, 1)))   # [128, T]
    sin4 = np.ascontiguousarray(np.tile(sinT, (4, 1)))
    ones1 = np.ones((1, 128), BF)

    groups = []
    for g in range(2):
        heads = [g * HPG + i for i in range(HPG)]
        qk_cols = []
        for p in range(NPAIR):
            A, Bh = heads[2 * p], heads[2 * p + 1]
            for base in (0, DK):                  # q block then k block
                for h in (A, Bh):
                    qk_cols += list(3 * DK * h + base + np.arange(0, DK, 2))
                    qk_cols += list(3 * DK * h + base + np.arange(1, DK, 2))
        qk_cols = np.array(qk_cols)
        wqk = np.ascontiguousarray(W_qkv[:, qk_cols].astype(BF))      # [1024, 1024]
        bqk = np.ascontiguousarray(b_qkv[qk_cols].reshape(8, 128).T)  # [128, 8]
        # v with interleaved ones cols, pair-major: [1024, 8*65]
        wva = np.zeros((D, 520), np.float32)
        bva = np.zeros((1, 520), np.float32)
        for i, h in enumerate(heads):
            vcols = 3 * DK * h + 2 * DK + np.arange(DK)
            wva[:, i * 65:i * 65 + 64] = W_qkv[:, vcols]
            bva[0, i * 65:i * 65 + 64] = b_qkv[vcols]
            bva[0, i * 65 + 64] = 1.0                 # ones column
        wo = np.ascontiguousarray(W_out[g * 512:(g + 1) * 512, :].astype(BF))
        groups.append(dict(wqk=wqk, bqk=bqk,
                           wva=np.ascontiguousarray(wva.astype(BF)),
                           bva=np.ascontiguousarray(bva.astype(BF)), wo=wo))

    in_maps = []
    for c in range(NC_):
        b, g = c // 2, c % 2
        gr = groups[g]
        in_maps.append({
            "xT": xTs[b], "wqk": gr["wqk"], "wva": gr["wva"], "bva": gr["bva"],
            "ones1": ones1, "wo": gr["wo"], "bqk": gr["bqk"],
            "cos4": cos4, "sin4": sin4,
        })
    return in_maps


def run(x, W_qkv, b_qkv, W_out, b_out, cos, sin, trace=False, trace_cores=None):
    """Build/compile (cached), run on 8 cores, return (out, BassKernelResults)."""
    if "nc" not in _cache:
        _cache["nc"] = _build_nc()
    nc = _cache["nc"]
    in_maps = _prep_inputs(x, W_qkv, b_qkv, W_out, cos, sin)
    kw = {}
    if trace:
        kw = dict(trace=True, trace_cores=trace_cores or [0])
    res = bass_utils.run_bass_kernel_spmd(nc, in_maps, core_ids=list(range(NC_)), **kw)
    b_out = np.asarray(b_out, dtype=np.float32)
    out = np.empty((B, T, D), np.float32)
    for b in range(B):
        out[b] = res.results[2 * b]["out"] + res.results[2 * b + 1]["out"] + b_out[None, :]
    return out, res


def kernel(x, W_qkv, b_qkv, W_out, b_out, cos, sin):
    out, _ = run(x, W_qkv, b_qkv, W_out, b_out, cos, sin)
    return out
